# revision 3
# baseline (speedup 1.0000x reference)
"""Stress-majorization loss kernel for Trainium2 (8 NeuronCores).

Problem: pos [8192,2] f32, dist [8192,8192] f32 ->
    scalar sum of ((|p_i - p_j| - d_ij)/d_ij)^2 over entries with d_ij != 0.

Strategy (per-core row sharding, 1024 rows each):
 - Identity: sum((w-1)^2) = sum(w^2) - 2*sum(w) + count, with
   w_ij = pred_ij / d_ij and w^2 = sq_ij * q_ij for q = 1/d^2.
   Host sends q as bf16 (halving HBM traffic vs f32 dist), with q=0 on
   masked (d==0) entries so they contribute 0 to both device sums; the
   count of unmasked entries is added on the host.
 - sq_ij = |p_i - p_j|^2 + EPS as a K=12 bf16 matmul:
     a_i = [1, n_i+EPS, -2x_i, -2y_i],  b_j = [n_j, 1, x_j, y_j]
   with each fp32 component split into 2 bf16 terms; 3 dominant
   term-pairs kept (error ~5e-5 absolute; EPS=1.5e-4 keeps PSUM sq>0).
 - Device, per [128,8192] row-tile, pipelined at [128,2048] chunk grain:
     DMA: q chunk (0.5MB bf16)
     PE:  sq -> PSUM (4 matmuls of 512 cols, K=12 bf16)
     DVE: t = sq * q with fused accum (tensor_tensor_reduce) -> sum(t)
     ACT: sqrt(t) over the full row-tile with accum -> sum(sqrt(t))
   Single DVE pass + single ACT pass per element (baseline had 2+2).
 - Host: total = sum(t) - 2*sum(sqrt(t)) + (N^2 - #zeros).
"""
import sys
sys.path.insert(0, "/opt/trn_rl_repo")

import numpy as np
import ml_dtypes

N = 8192
NCORES = 8
ROWS_PER_CORE = N // NCORES          # 1024
RTILES = ROWS_PER_CORE // 128        # 8 row tiles of 128
CHUNK = 2048                         # PSUM chunk (4 banks)
MMF = 512                            # matmul free dim (1 PSUM bank)
KB = 4                               # base contraction dim
NPAIR = 3                            # bf16 split term-pairs kept
K = KB * NPAIR                       # 12
NCH = N // CHUNK                     # 4 chunks per row tile
EPS = np.float32(1.5e-4)             # keeps PSUM sq > 0 despite cancellation

_cache = {}


def _build_nc():
    import concourse.bacc as bacc
    import concourse.mybir as mybir
    import concourse.tile as tile

    f32 = mybir.dt.float32
    bf16 = mybir.dt.bfloat16
    A = mybir.ActivationFunctionType
    OP = mybir.AluOpType

    nc = bacc.Bacc("TRN2", target_bir_lowering=False, debug=False)
    qmat = nc.dram_tensor("qmat", [ROWS_PER_CORE, N], bf16, kind="ExternalInput")
    acore = nc.dram_tensor("acore", [K, ROWS_PER_CORE], bf16, kind="ExternalInput")
    bfull = nc.dram_tensor("bfull", [K, N], bf16, kind="ExternalInput")
    # out columns: [0:RTILES*NCH) = per-chunk sum(t), [RTILES*NCH:+RTILES) = per-tile sum(sqrt(t))
    out = nc.dram_tensor("out", [128, RTILES * NCH + RTILES], f32,
                         kind="ExternalOutput")

    with tile.TileContext(nc) as tc:
        with tc.tile_pool(name="small", bufs=1) as small, \
             tc.tile_pool(name="qpool", bufs=6) as qpool, \
             tc.tile_pool(name="tpool", bufs=2) as tpool, \
             tc.tile_pool(name="psum", bufs=2, space="PSUM") as psp:

            t_a = small.tile([K, ROWS_PER_CORE], bf16)
            t_b = small.tile([K, N], bf16)
            t_acct = small.tile([128, RTILES * NCH], f32)
            t_accv = small.tile([128, RTILES], f32)
            t_v = small.tile([128, N], bf16)       # sqrt scratch, never read
            nc.sync.dma_start(t_a[:], acore[:])
            nc.sync.dma_start(t_b[:], bfull[:])

            for r in range(RTILES):
                lhsT = t_a[:, r * 128:(r + 1) * 128]
                t_t = tpool.tile([128, N], f32, tag="t")
                for q in range(NCH):
                    c0 = q * CHUNK
                    t_q = qpool.tile([128, CHUNK], bf16, tag="q")
                    nc.sync.dma_start(
                        t_q[:], qmat[r * 128:(r + 1) * 128, c0:c0 + CHUNK])
                    t_ps = psp.tile([128, CHUNK], f32, tag="ps")
                    for j in range(CHUNK // MMF):
                        col = c0 + j * MMF
                        nc.tensor.matmul(
                            t_ps[:, j * MMF:(j + 1) * MMF],
                            lhsT,
                            t_b[:, col:col + MMF],
                            start=True, stop=True)
                    # t = sq * q, with fused per-partition sum(t) accumulation
                    # (scalar_tensor_tensor: tensor_tensor_reduce traps on HW)
                    nc.vector.scalar_tensor_tensor(
                        out=t_t[:, c0:c0 + CHUNK],
                        in0=t_ps[:],
                        scalar=0.0,
                        in1=t_q[:],
                        op0=OP.bypass,
                        op1=OP.mult,
                        accum_out=t_acct[:, r * NCH + q:r * NCH + q + 1])
                # v = sqrt(t) over the whole row tile; only the accum matters
                nc.scalar.activation(
                    t_v[:], t_t[:], A.Sqrt,
                    accum_out=t_accv[:, r:r + 1])

            nc.sync.dma_start(out[:, 0:RTILES * NCH], t_acct[:])
            nc.sync.dma_start(out[:, RTILES * NCH:], t_accv[:])

    nc.compile()
    return nc


def _split2(v: np.ndarray):
    """Split fp32 vector into 2 bf16 terms summing to v (error ~2^-18 |v|)."""
    v = v.astype(np.float32)
    v0 = v.astype(ml_dtypes.bfloat16)
    r1 = v - v0.astype(np.float32)
    v1 = r1.astype(ml_dtypes.bfloat16)
    return v0, v1


def _to_np_f32(x):
    try:
        return np.ascontiguousarray(x, dtype=np.float32)
    except Exception:
        import jax
        return np.ascontiguousarray(jax.device_get(x), dtype=np.float32)


def _prep_inputs(pos: np.ndarray, dist: np.ndarray):
    pos = _to_np_f32(pos)
    dist = _to_np_f32(dist)
    assert pos.shape == (N, 2) and dist.shape == (N, N)

    # q = 1/d^2 in bf16; q=0 on masked (d==0) entries so they contribute 0
    zmask = dist == 0.0
    nzeros = int(np.count_nonzero(zmask))
    dist_safe = np.where(zmask, np.float32(1.0), dist)
    q = (np.float32(1.0) / (dist_safe * dist_safe)).astype(ml_dtypes.bfloat16)
    q[zmask] = ml_dtypes.bfloat16(0.0)

    x = pos[:, 0].astype(np.float64)
    y = pos[:, 1].astype(np.float64)
    n = x * x + y * y
    a_full32 = np.stack([np.ones(N), n + np.float64(EPS), -2.0 * x, -2.0 * y]
                        ).astype(np.float32)          # [4, N]
    b_full32 = np.stack([n, np.ones(N), x, y]).astype(np.float32)  # [4, N]

    a0, a1 = _split2(a_full32)
    b0, b1 = _split2(b_full32)
    # term pairs kept: (a0,b0) (a0,b1) (a1,b0)
    a_parts = [a0, a0, a1]
    b_parts = [b0, b1, b0]
    a_full = np.concatenate(a_parts, axis=0)   # [12, N] bf16
    b_full = np.concatenate(b_parts, axis=0)   # [12, N] bf16

    in_maps = []
    for c in range(NCORES):
        r0 = c * ROWS_PER_CORE
        in_maps.append({
            "qmat": q[r0:r0 + ROWS_PER_CORE, :],
            "acore": np.ascontiguousarray(a_full[:, r0:r0 + ROWS_PER_CORE]),
            "bfull": b_full,
        })
    return in_maps, nzeros


def kernel(pos: np.ndarray, dist: np.ndarray) -> np.ndarray:
    from concourse.bass_utils import run_bass_kernel_spmd

    in_maps, nzeros = _prep_inputs(pos, dist)
    if "nc" not in _cache:
        _cache["nc"] = _build_nc()
    nc = _cache["nc"]

    res = run_bass_kernel_spmd(nc, in_maps, list(range(NCORES)))
    sum_t = 0.0
    sum_v = 0.0
    for c in range(NCORES):
        o = res.results[c]["out"].astype(np.float64)
        sum_t += o[:, :RTILES * NCH].sum()
        sum_v += o[:, RTILES * NCH:].sum()
    total = sum_t - 2.0 * sum_v + float(N * N - nzeros)
    return np.array(total, dtype=np.float32)
